# revision 7
# baseline (speedup 1.0000x reference)
"""Trainium2 Bass kernel for nn_DenSparseMatrix (gnn_message_passing).

Math: out[b, o] = sum_k rm[o,k] * s[idx[o,k], k] * x[b, idx[o,k]],
      s = forward_weights * forward_mask  (elementwise, [I, W])

Strategy (8 NeuronCores, SPMD):
  * Shard output rows: core c owns o in [c*8192, (c+1)*8192).
  * bf16 gather table with 256B rows pairing two inputs:
    T[j] = [xT[2j] | s[2j] | xT[2j+1] | s[2j+1]] (each 32 bf16).
  * Tokens with exactly-zero coefficient (reverse_mask==0 or
    forward_mask[idx]==0) are pruned on the host; each output's alive
    tokens are compacted into S slots (~75% pruned).  This cuts the
    per-token SWDGE descriptor-generation and DMA packet-rate cost,
    which are the kernel's limiting resources.
  * Per 256-output pair-block: one gather of 128*2S rows; token
    t = (s*2+bi)*128 + p -> partition p (output o%128), slot s*2+bi.
  * Coefficient: a host-built one-hot mask M[token, 64] (rm baked in)
    selects s[i,k] from the gathered row's two s-halves via one
    contiguous multiply + reduce_sum; a parity mask splits the result
    into the (even, odd) x-half coefficients; then one contiguous
    multiply into tmp and a log-tree fold reduction.
"""

import numpy as np
import ml_dtypes

import concourse.bass as bass
import concourse.bacc as bacc
import concourse.mybir as mybir
from concourse.tile import TileContext
from concourse.bass_utils import run_bass_kernel_spmd
from concourse.library_config import mlp

I = 65536
O = 65536
W = 32
B = 32
NCORES = 8
O_SHARD = O // NCORES        # 8192 outputs per core
NBP = O_SHARD // 256         # 32 pair-blocks of 256 outputs
ROW = 4 * W                  # 128 bf16 per table row (256B)
NQ = 4
F32 = mybir.dt.float32
BF16 = mybir.dt.bfloat16
I16 = mybir.dt.int16
BF16_NP = ml_dtypes.bfloat16

ROWS_PER_PART = I // 128     # 512
NT = 8
ROWS_PER_TILE = ROWS_PER_PART // NT  # 64

S_DEFAULT = 20               # compacted slots per output


def _build_nc(S):
    NS = 2 * S               # slots per partition per pair-block
    NIDX = 128 * NS
    IDXF = NIDX // 16
    nc = bacc.Bacc("TRN2", target_bir_lowering=False, debug=False,
                   num_devices=NCORES, num_swdge_queues=NQ)

    xT_d = nc.dram_tensor("xT", [128, ROWS_PER_PART * B], BF16, kind="ExternalInput")
    fw_d = nc.dram_tensor("fw", [128, ROWS_PER_PART * W], BF16, kind="ExternalInput")
    fm_d = nc.dram_tensor("fm", [128, ROWS_PER_PART * W], BF16, kind="ExternalInput")
    idx_d = nc.dram_tensor("idx", [128, NBP * IDXF], I16, kind="ExternalInput")
    M_d = nc.dram_tensor("M", [128, NBP * NS * 64], BF16, kind="ExternalInput")
    hm_d = nc.dram_tensor("hm", [128, NBP * NS * 2], BF16, kind="ExternalInput")
    out_d = nc.dram_tensor("out", [128, NBP * 2 * B], F32, kind="ExternalOutput")
    tab_d = nc.dram_tensor("tab", [I // 2, ROW], BF16, kind="Internal")

    tab_pv = tab_d[:, :].rearrange("(p a) b -> p (a b)", p=128)

    with TileContext(nc) as tc:
        nc.gpsimd.load_library(mlp)

        # ---- Phase 1: build the packed table in HBM -------------------
        with (
            tc.tile_pool(name="p1in", bufs=2) as p1in,
            tc.tile_pool(name="p1st", bufs=2) as p1st,
        ):
            npt = ROWS_PER_TILE * B
            for t in range(NT):
                xt = p1in.tile([128, ROWS_PER_TILE, B], BF16, tag="xt")
                nc.sync.dma_start(
                    xt[:], xT_d[:, t * npt:(t + 1) * npt].rearrange(
                        "p (a b) -> p a b", b=B))
                fwt = p1in.tile([128, ROWS_PER_TILE, W], BF16, tag="fwt")
                nc.sync.dma_start(
                    fwt[:], fw_d[:, t * npt:(t + 1) * npt].rearrange(
                        "p (a b) -> p a b", b=W))
                fmt = p1in.tile([128, ROWS_PER_TILE, W], BF16, tag="fmt")
                nc.sync.dma_start(
                    fmt[:], fm_d[:, t * npt:(t + 1) * npt].rearrange(
                        "p (a b) -> p a b", b=W))
                stage = p1st.tile([128, ROWS_PER_TILE, 2 * B], BF16, tag="stage")
                nc.vector.tensor_copy(stage[:, :, 0:B], xt[:])
                nc.vector.tensor_mul(stage[:, :, B:2 * B], fwt[:], fmt[:])
                nc.sync.dma_start(
                    tab_pv[:, t * ROWS_PER_TILE * 2 * B:(t + 1) * ROWS_PER_TILE * 2 * B],
                    stage[:].rearrange("p a b -> p (a b)"))

        # ---- Phase 2: gather + masked reduce per pair-block -----------
        with (
            tc.tile_pool(name="pres", bufs=1) as pres,
            tc.tile_pool(name="pg", bufs=4) as pg,
            tc.tile_pool(name="pm", bufs=3) as pm,
            tc.tile_pool(name="psm", bufs=4) as psm,
            tc.tile_pool(name="ptmp", bufs=3) as ptmp,
        ):
            idx_all = pres.tile([128, NBP * IDXF], I16)
            nc.sync.dma_start(idx_all[:], idx_d[:])
            hm_all = pres.tile([128, NBP * NS * 2], BF16)
            nc.sync.dma_start(hm_all[:], hm_d[:])
            ocore = pres.tile([128, NBP * 2 * B], F32)

            for bp in range(NBP):
                G = pg.tile([128, NS, ROW], BF16, tag="G")
                nc.gpsimd.dma_gather(
                    G[:], tab_d[:, :],
                    idx_all[:, bp * IDXF:(bp + 1) * IDXF],
                    NIDX, NIDX, ROW, single_packet=False, queue_num=bp % NQ)
                Mt = pm.tile([128, NS * 64], BF16, tag="Mt")
                nc.sync.dma_start(
                    Mt[:], M_d[:, bp * NS * 64:(bp + 1) * NS * 64])

                gap = G[:]
                # P[p, slot, h', k'] = G[p, slot, 32 + 64h' + k'] * M
                gs = bass.AP(gap.tensor, gap.offset + B,
                             [list(gap.ap[0]), [ROW, NS], [2 * B, 2], [1, B]])
                mv = Mt[:]
                m_ap = bass.AP(mv.tensor, mv.offset,
                               [list(mv.ap[0]), [64, NS], [B, 2], [1, B]])
                P = psm.tile([128, NS, 64], BF16, tag="P")
                pv = P[:]
                p_ap = bass.AP(pv.tensor, pv.offset,
                               [list(pv.ap[0]), [64, NS], [B, 2], [1, B]])
                nc.vector.tensor_mul(p_ap, gs, m_ap)

                # c[p, slot] = sum over 64 (f32)
                c = psm.tile([128, NS], F32, tag="c")
                nc.vector.reduce_sum(c[:], P[:], axis=mybir.AxisListType.X)

                # c01[p, slot, h] = c * parity mask
                c01 = psm.tile([128, NS * 2], BF16, tag="c01")
                cv, hv, c01v = c[:], hm_all[:], c01[:]
                c_ap = bass.AP(cv.tensor, cv.offset,
                               [list(cv.ap[0]), [1, NS], [0, 2]])
                h_ap = bass.AP(hv.tensor, hv.offset + bp * NS * 2,
                               [list(hv.ap[0]), [2, NS], [1, 2]])
                c01_ap = bass.AP(c01v.tensor, c01v.offset,
                                 [list(c01v.ap[0]), [2, NS], [1, 2]])
                nc.vector.tensor_mul(c01_ap, c_ap, h_ap)

                # tmp[p, slot, h, b] = G[p, slot, 64h + b] * c01[p, slot, h]
                tmp = ptmp.tile([128, NS * 64], BF16, tag="tmp")
                gx = bass.AP(gap.tensor, gap.offset,
                             [list(gap.ap[0]), [ROW, NS], [2 * B, 2], [1, B]])
                ab = bass.AP(c01v.tensor, c01v.offset,
                             [list(c01v.ap[0]), [2, NS], [1, 2], [0, B]])
                tv = tmp[:]
                t_ap = bass.AP(tv.tensor, tv.offset,
                               [list(tv.ap[0]), [64, NS], [B, 2], [1, B]])
                nc.vector.tensor_mul(t_ap, gx, ab)

                # fold over s (slot-major pairs of 128-elem groups): S -> 1
                n = S
                while n > 1:
                    half = n // 2
                    nc.vector.tensor_add(
                        tmp[:, 0:half * 128], tmp[:, 0:half * 128],
                        tmp[:, half * 128:2 * half * 128])
                    if n % 2:
                        nc.vector.tensor_add(
                            tmp[:, 0:128], tmp[:, 0:128],
                            tmp[:, (n - 1) * 128:n * 128])
                    n = half
                # remaining [bi, h, b]; h-fold into f32 ocore[bi, b]
                oc = ocore[:, bp * 2 * B:(bp + 1) * 2 * B]
                in0 = bass.AP(tv.tensor, tv.offset,
                              [list(tv.ap[0]), [2 * B, 2], [1, B]])
                in1 = bass.AP(tv.tensor, tv.offset + B,
                              [list(tv.ap[0]), [2 * B, 2], [1, B]])
                o_ap = bass.AP(oc.tensor, oc.offset,
                               [list(oc.ap[0]), [B, 2], [1, B]])
                nc.vector.tensor_add(o_ap, in0, in1)

            nc.sync.dma_start(out_d[:], ocore[:])

    nc.compile()
    return nc


_NC = {}


def _get_nc(S):
    if S not in _NC:
        _NC[S] = _build_nc(S)
    return _NC[S]


def _prep(x, forward_weights, forward_mask, output_mapping, reverse_mask):
    x = np.asarray(x, dtype=np.float32)
    fw = np.ascontiguousarray(np.asarray(forward_weights, dtype=np.float32))
    fm = np.ascontiguousarray(np.asarray(forward_mask, dtype=np.float32))
    idx = np.asarray(output_mapping).astype(np.int64)
    rm = np.asarray(reverse_mask, dtype=np.float32)
    kcol = np.arange(W)[None, :]
    alive = (rm != 0) & (fm[idx, kcol] != 0)    # [O, W]
    S = max(S_DEFAULT, int(alive.sum(1).max()))
    return x, fw, fm, idx, rm, alive, S


def make_in_maps(x, fw, fm, idx, rm, alive, S):
    NS = 2 * S
    NIDX = 128 * NS
    IDXF = NIDX // 16

    xT_v = np.ascontiguousarray(x.T).astype(BF16_NP).reshape(128, ROWS_PER_PART * B)
    fw_v = fw.astype(BF16_NP).reshape(128, ROWS_PER_PART * W)
    fm_v = fm.astype(BF16_NP).reshape(128, ROWS_PER_PART * W)

    in_maps = []
    for c in range(NCORES):
        sh = slice(c * O_SHARD, (c + 1) * O_SHARD)
        idx_c, rm_c, al_c = idx[sh], rm[sh], alive[sh]
        order = np.argsort(~al_c, axis=1, kind="stable")[:, :S]   # [8192, S]
        a_s = np.take_along_axis(al_c, order, axis=1)
        k_s = order
        i_s = np.take_along_axis(idx_c, order, axis=1)
        rm_s = np.take_along_axis(rm_c, order, axis=1) * a_s
        j_s = np.where(a_s, i_s >> 1, 0).astype(np.int16)         # pad -> row 0
        h_s = (i_s & 1).astype(np.int64)

        rows = np.arange(O_SHARD)[:, None]
        cols = np.arange(S)[None, :]
        M = np.zeros((O_SHARD, S, 64), np.float32)
        M[rows, cols, h_s * 32 + k_s] = rm_s
        hmv = np.zeros((O_SHARD, S, 2), np.float32)
        hmv[rows, cols, h_s] = a_s.astype(np.float32)

        # token t = (s*2+bi)*128 + p
        jb = j_s.reshape(NBP, 2, 128, S)                   # [bp, bi, p, s]
        L = jb.transpose(0, 3, 1, 2).reshape(NBP, NIDX)    # [bp, (s,bi,p)]
        idx_w = L.reshape(NBP, IDXF, 16).transpose(0, 2, 1)
        idx_w = np.tile(idx_w, (1, 8, 1))
        idx_all = np.ascontiguousarray(
            idx_w.transpose(1, 0, 2).reshape(128, NBP * IDXF))
        M_all = np.ascontiguousarray(
            M.reshape(NBP, 2, 128, S, 64).transpose(2, 0, 3, 1, 4)
            .reshape(128, NBP * NS * 64)).astype(BF16_NP)
        hm_all = np.ascontiguousarray(
            hmv.reshape(NBP, 2, 128, S, 2).transpose(2, 0, 3, 1, 4)
            .reshape(128, NBP * NS * 2)).astype(BF16_NP)
        in_maps.append({
            "xT": xT_v, "fw": fw_v, "fm": fm_v,
            "idx": idx_all, "M": M_all, "hm": hm_all,
        })
    return in_maps


def unshard_out(results):
    out = np.empty((B, O), np.float32)
    for c in range(NCORES):
        oc = results[c]["out"]              # [128, NBP*2*B]
        out[:, c * O_SHARD:(c + 1) * O_SHARD] = (
            oc.reshape(128, NBP, 2, B).transpose(3, 1, 2, 0).reshape(B, O_SHARD)
        )
    return out


def kernel(x, forward_weights, forward_mask, output_mapping, reverse_mask):
    x, fw, fm, idx, rm, alive, S = _prep(
        x, forward_weights, forward_mask, output_mapping, reverse_mask)
    nc = _get_nc(S)
    in_maps = make_in_maps(x, fw, fm, idx, rm, alive, S)
    res = run_bass_kernel_spmd(nc, in_maps, core_ids=list(range(NCORES)))
    return unshard_out(res.results)


# revision 8
# speedup vs baseline: 2.2992x; 2.2992x over previous
"""Trainium2 Bass kernel for nn_DenSparseMatrix (gnn_message_passing).

Math: out[b, o] = sum_k rm[o,k] * s[idx[o,k], k] * x[b, idx[o,k]],
      s = forward_weights * forward_mask  (elementwise, [I, W])

Strategy (8 NeuronCores, SPMD):
  * Shard output rows: core c owns o in [c*8192, (c+1)*8192).
  * bf16 gather table with 256B rows pairing two inputs:
    T[j] = [xT[2j] | s[2j] | xT[2j+1] | s[2j+1]] (each 32 bf16).
  * Tokens with exactly-zero coefficient (reverse_mask==0 or
    forward_mask[idx]==0) are pruned on the host; each output's alive
    tokens are compacted into S slots (~75% pruned).  This cuts the
    per-token SWDGE descriptor-generation and DMA packet-rate cost,
    which are the kernel's limiting resources.
  * Per 256-output pair-block: one gather of 128*2S rows; token
    t = (s*2+bi)*128 + p -> partition p (output o%128), slot s*2+bi.
  * Coefficient: a host-built one-hot mask M[token, 64] (rm baked in)
    selects s[i,k] from the gathered row's two s-halves via one
    contiguous multiply + reduce_sum; a parity mask splits the result
    into the (even, odd) x-half coefficients; then one contiguous
    multiply into tmp and a log-tree fold reduction.
"""

import numpy as np
import ml_dtypes

import concourse.bass as bass
import concourse.bacc as bacc
import concourse.mybir as mybir
from concourse.tile import TileContext
from concourse.bass_utils import run_bass_kernel_spmd
from concourse.library_config import mlp

I = 65536
O = 65536
W = 32
B = 32
NCORES = 8
O_SHARD = O // NCORES        # 8192 outputs per core
NBP = O_SHARD // 256         # 32 pair-blocks of 256 outputs
ROW = 4 * W                  # 128 bf16 per table row (256B)
NQ = 4
F32 = mybir.dt.float32
BF16 = mybir.dt.bfloat16
I16 = mybir.dt.int16
BF16_NP = ml_dtypes.bfloat16

ROWS_PER_PART = I // 128     # 512
NT = 8
ROWS_PER_TILE = ROWS_PER_PART // NT  # 64

S_DEFAULT = 20               # compacted slots per output


def _build_nc(S):
    NS = 2 * S               # slots per partition per pair-block
    NIDX = 128 * NS
    IDXF = NIDX // 16
    nc = bacc.Bacc("TRN2", target_bir_lowering=False, debug=False,
                   num_devices=NCORES, num_swdge_queues=NQ)

    xT_d = nc.dram_tensor("xT", [128, ROWS_PER_PART * B], BF16, kind="ExternalInput")
    fw_d = nc.dram_tensor("fw", [128, ROWS_PER_PART * W], BF16, kind="ExternalInput")
    fm_d = nc.dram_tensor("fm", [128, ROWS_PER_PART * W], BF16, kind="ExternalInput")
    idx_d = nc.dram_tensor("idx", [128, NBP * IDXF], I16, kind="ExternalInput")
    M_d = nc.dram_tensor("M", [128, NBP * NS * 64], BF16, kind="ExternalInput")
    hm_d = nc.dram_tensor("hm", [128, NBP * NS * 2], BF16, kind="ExternalInput")
    out_d = nc.dram_tensor("out", [128, NBP * 2 * B], F32, kind="ExternalOutput")
    tab_d = nc.dram_tensor("tab", [I // 2, ROW], BF16, kind="Internal")

    tab_pv = tab_d[:, :].rearrange("(p a) b -> p (a b)", p=128)

    with TileContext(nc) as tc:
        nc.gpsimd.load_library(mlp)

        # ---- Phase 1: build the packed table in HBM -------------------
        with (
            tc.tile_pool(name="p1in", bufs=2) as p1in,
            tc.tile_pool(name="p1st", bufs=2) as p1st,
        ):
            npt = ROWS_PER_TILE * B
            for t in range(NT):
                xt = p1in.tile([128, ROWS_PER_TILE, B], BF16, tag="xt")
                nc.sync.dma_start(
                    xt[:], xT_d[:, t * npt:(t + 1) * npt].rearrange(
                        "p (a b) -> p a b", b=B))
                fwt = p1in.tile([128, ROWS_PER_TILE, W], BF16, tag="fwt")
                nc.sync.dma_start(
                    fwt[:], fw_d[:, t * npt:(t + 1) * npt].rearrange(
                        "p (a b) -> p a b", b=W))
                fmt = p1in.tile([128, ROWS_PER_TILE, W], BF16, tag="fmt")
                nc.sync.dma_start(
                    fmt[:], fm_d[:, t * npt:(t + 1) * npt].rearrange(
                        "p (a b) -> p a b", b=W))
                stage = p1st.tile([128, ROWS_PER_TILE, 2 * B], BF16, tag="stage")
                nc.vector.tensor_copy(stage[:, :, 0:B], xt[:])
                nc.vector.tensor_mul(stage[:, :, B:2 * B], fwt[:], fmt[:])
                nc.sync.dma_start(
                    tab_pv[:, t * ROWS_PER_TILE * 2 * B:(t + 1) * ROWS_PER_TILE * 2 * B],
                    stage[:].rearrange("p a b -> p (a b)"))

        # ---- Phase 2: gather + masked reduce per pair-block -----------
        with (
            tc.tile_pool(name="pres", bufs=1) as pres,
            tc.tile_pool(name="pg", bufs=4) as pg,
            tc.tile_pool(name="pm", bufs=3) as pm,
            tc.tile_pool(name="psm", bufs=4) as psm,
            tc.tile_pool(name="ptmp", bufs=3) as ptmp,
        ):
            idx_all = pres.tile([128, NBP * IDXF], I16)
            nc.sync.dma_start(idx_all[:], idx_d[:])
            hm_all = pres.tile([128, NBP * NS * 2], BF16)
            nc.sync.dma_start(hm_all[:], hm_d[:])
            ocore = pres.tile([128, NBP * 2 * B], F32)

            for bp in range(NBP):
                G = pg.tile([128, NS, ROW], BF16, tag="G")
                nc.gpsimd.dma_gather(
                    G[:], tab_d[:, :],
                    idx_all[:, bp * IDXF:(bp + 1) * IDXF],
                    NIDX, NIDX, ROW, single_packet=False, queue_num=bp % NQ)
                Mt = pm.tile([128, NS * 64], BF16, tag="Mt")
                nc.sync.dma_start(
                    Mt[:], M_d[:, bp * NS * 64:(bp + 1) * NS * 64])

                gap = G[:]
                # P[p, slot, h', k'] = G[p, slot, 32 + 64h' + k'] * M
                gs = bass.AP(gap.tensor, gap.offset + B,
                             [list(gap.ap[0]), [ROW, NS], [2 * B, 2], [1, B]])
                mv = Mt[:]
                m_ap = bass.AP(mv.tensor, mv.offset,
                               [list(mv.ap[0]), [64, NS], [B, 2], [1, B]])
                P = psm.tile([128, NS, 64], BF16, tag="P")
                pv = P[:]
                p_ap = bass.AP(pv.tensor, pv.offset,
                               [list(pv.ap[0]), [64, NS], [B, 2], [1, B]])
                nc.vector.tensor_mul(p_ap, gs, m_ap)

                # c[p, slot] = sum over 64 (f32)
                c = psm.tile([128, NS], F32, tag="c")
                nc.vector.reduce_sum(c[:], P[:], axis=mybir.AxisListType.X)

                # c01[p, slot, h] = c * parity mask
                c01 = psm.tile([128, NS * 2], BF16, tag="c01")
                cv, hv, c01v = c[:], hm_all[:], c01[:]
                c_ap = bass.AP(cv.tensor, cv.offset,
                               [list(cv.ap[0]), [1, NS], [0, 2]])
                h_ap = bass.AP(hv.tensor, hv.offset + bp * NS * 2,
                               [list(hv.ap[0]), [2, NS], [1, 2]])
                c01_ap = bass.AP(c01v.tensor, c01v.offset,
                                 [list(c01v.ap[0]), [2, NS], [1, 2]])
                nc.vector.tensor_mul(c01_ap, c_ap, h_ap)

                # tmp[p, slot, h, b] = G[p, slot, 64h + b] * c01[p, slot, h]
                tmp = ptmp.tile([128, NS * 64], BF16, tag="tmp")
                gx = bass.AP(gap.tensor, gap.offset,
                             [list(gap.ap[0]), [ROW, NS], [2 * B, 2], [1, B]])
                ab = bass.AP(c01v.tensor, c01v.offset,
                             [list(c01v.ap[0]), [2, NS], [1, 2], [0, B]])
                tv = tmp[:]
                t_ap = bass.AP(tv.tensor, tv.offset,
                               [list(tv.ap[0]), [64, NS], [B, 2], [1, B]])
                nc.vector.tensor_mul(t_ap, gx, ab)

                # fold over s (slot-major pairs of 128-elem groups): S -> 1
                n = S
                while n > 1:
                    half = n // 2
                    nc.vector.tensor_add(
                        tmp[:, 0:half * 128], tmp[:, 0:half * 128],
                        tmp[:, half * 128:2 * half * 128])
                    if n % 2:
                        nc.vector.tensor_add(
                            tmp[:, 0:128], tmp[:, 0:128],
                            tmp[:, (n - 1) * 128:n * 128])
                    n = half
                # remaining [bi, h, b]; h-fold into f32 ocore[bi, b]
                oc = ocore[:, bp * 2 * B:(bp + 1) * 2 * B]
                in0 = bass.AP(tv.tensor, tv.offset,
                              [list(tv.ap[0]), [2 * B, 2], [1, B]])
                in1 = bass.AP(tv.tensor, tv.offset + B,
                              [list(tv.ap[0]), [2 * B, 2], [1, B]])
                o_ap = bass.AP(oc.tensor, oc.offset,
                               [list(oc.ap[0]), [B, 2], [1, B]])
                nc.vector.tensor_add(o_ap, in0, in1)

            nc.sync.dma_start(out_d[:], ocore[:])

    nc.compile()
    return nc


_NC = {}


def _get_nc(S):
    if S not in _NC:
        _NC[S] = _build_nc(S)
    return _NC[S]


def _prep(x, forward_weights, forward_mask, output_mapping, reverse_mask):
    x = np.asarray(x, dtype=np.float32)
    fw = np.ascontiguousarray(np.asarray(forward_weights, dtype=np.float32))
    fm = np.ascontiguousarray(np.asarray(forward_mask, dtype=np.float32))
    idx = np.asarray(output_mapping).astype(np.int64)
    rm = np.asarray(reverse_mask, dtype=np.float32)
    kcol = np.arange(W)[None, :]
    alive = (rm != 0) & (fm[idx, kcol] != 0)    # [O, W]
    S = max(S_DEFAULT, int(alive.sum(1).max()))
    return x, fw, fm, idx, rm, alive, S


def make_in_maps(x, fw, fm, idx, rm, alive, S):
    NS = 2 * S
    NIDX = 128 * NS
    IDXF = NIDX // 16

    xT_v = np.ascontiguousarray(x.T).astype(BF16_NP).reshape(128, ROWS_PER_PART * B)
    fw_v = fw.astype(BF16_NP).reshape(128, ROWS_PER_PART * W)
    fm_v = fm.astype(BF16_NP).reshape(128, ROWS_PER_PART * W)

    in_maps = []
    for c in range(NCORES):
        sh = slice(c * O_SHARD, (c + 1) * O_SHARD)
        idx_c, rm_c, al_c = idx[sh], rm[sh], alive[sh]
        order = np.argsort(~al_c, axis=1, kind="stable")[:, :S]   # [8192, S]
        a_s = np.take_along_axis(al_c, order, axis=1)
        k_s = order
        i_s = np.take_along_axis(idx_c, order, axis=1)
        rm_s = np.take_along_axis(rm_c, order, axis=1) * a_s
        rows0 = np.arange(O_SHARD)[:, None]
        cols0 = np.arange(S)[None, :]
        spread = ((rows0 * S + cols0) * 2654435761) % (I // 2)
        j_s = np.where(a_s, i_s >> 1, spread).astype(np.int16)    # scatter pads
        h_s = (i_s & 1).astype(np.int64)

        rows = np.arange(O_SHARD)[:, None]
        cols = np.arange(S)[None, :]
        M = np.zeros((O_SHARD, S, 64), np.float32)
        M[rows, cols, h_s * 32 + k_s] = rm_s
        hmv = np.zeros((O_SHARD, S, 2), np.float32)
        hmv[rows, cols, h_s] = a_s.astype(np.float32)

        # token t = (s*2+bi)*128 + p
        jb = j_s.reshape(NBP, 2, 128, S)                   # [bp, bi, p, s]
        L = jb.transpose(0, 3, 1, 2).reshape(NBP, NIDX)    # [bp, (s,bi,p)]
        idx_w = L.reshape(NBP, IDXF, 16).transpose(0, 2, 1)
        idx_w = np.tile(idx_w, (1, 8, 1))
        idx_all = np.ascontiguousarray(
            idx_w.transpose(1, 0, 2).reshape(128, NBP * IDXF))
        M_all = np.ascontiguousarray(
            M.reshape(NBP, 2, 128, S, 64).transpose(2, 0, 3, 1, 4)
            .reshape(128, NBP * NS * 64)).astype(BF16_NP)
        hm_all = np.ascontiguousarray(
            hmv.reshape(NBP, 2, 128, S, 2).transpose(2, 0, 3, 1, 4)
            .reshape(128, NBP * NS * 2)).astype(BF16_NP)
        in_maps.append({
            "xT": xT_v, "fw": fw_v, "fm": fm_v,
            "idx": idx_all, "M": M_all, "hm": hm_all,
        })
    return in_maps


def unshard_out(results):
    out = np.empty((B, O), np.float32)
    for c in range(NCORES):
        oc = results[c]["out"]              # [128, NBP*2*B]
        out[:, c * O_SHARD:(c + 1) * O_SHARD] = (
            oc.reshape(128, NBP, 2, B).transpose(3, 1, 2, 0).reshape(B, O_SHARD)
        )
    return out


def kernel(x, forward_weights, forward_mask, output_mapping, reverse_mask):
    x, fw, fm, idx, rm, alive, S = _prep(
        x, forward_weights, forward_mask, output_mapping, reverse_mask)
    nc = _get_nc(S)
    in_maps = make_in_maps(x, fw, fm, idx, rm, alive, S)
    res = run_bass_kernel_spmd(nc, in_maps, core_ids=list(range(NCORES)))
    return unshard_out(res.results)
